# revision 3
# baseline (speedup 1.0000x reference)
"""Trainium2 Bass kernel v7: conv2d(3x3, VALID) + bias -> channel-min -> tanh(tanh).

Strategy vs v6 baseline:
- Single fp8 input layout `px` [128, 130*128]: block b holds x row b on
  partitions 0:64 and x row b+1 on partitions 64:128, full 128-wide rows.
  Halves input DMA vs the v6 dual pab/pct layouts.
- Conv = 3 DoubleRow matmuls per 512-px tile (col offsets 0,1,2; row-shift
  planes stride 128). Per mm_c: taps (0,c),(1,c) on plane 0 and (2,c) on
  plane 1 (partitions 64:128); zero-padded slots elsewhere. 1.5 cyc/px on
  PE (vs 2.36 in v6); negated weights so channel-min becomes max.
- PSUM evac in 1536-px super-tiles (3 banks), split DVE (raw -(y+b),
  tensor_scalar_add) for the first DVE_TILES tiles / Act (tanh fused,
  monotone) for the rest.
- Channel-min: PE transposes + DVE max-tree for px 0..NCH*128 (waves of 8
  chunks interleaved into the conv stream); Pool partition_all_reduce for
  px NCH*128..16128 (junk rows excluded).
- Per-image finals (tanh) + output DMAs are emitted with a 2-image lag so
  they never head-of-line-block the next image's evacuations on Act; the
  pool-route scatter DMAs ride the gpsimd (SWDGE) queue, image loads ride
  the SP (HWDGE) queue.
- Output px space is 128x128 per image; cols/rows 126,127 are junk and
  cropped on host.
"""

import numpy as np

import concourse.bacc as bacc
import concourse.bass as bass
import concourse.bass_isa as bass_isa
import concourse.tile as tile
from concourse import mybir
from concourse.bass_utils import run_bass_kernel_spmd

N_CORES = 8
N_IMGS = 32
IMGS_PER_CORE = N_IMGS // N_CORES
H = W = 128
HO = WO = 126
HB = 128  # packed row-block width (full x row)
NROW = 128  # output px rows per image incl. 2 junk rows
PXS = NROW * HB  # 16384 px space per image
NBLK = 130  # row blocks (128 data + 2 zero)
PITCH = NBLK * HB  # 16640
# conv super-tiles: 10 x 1536 px + 1 x 1024 px (3/2 PSUM banks)
TILE_PX = [1536] * 10 + [1024]
TILE_OFF = [sum(TILE_PX[:i]) for i in range(len(TILE_PX))]
DVE_TILES = 3  # super-tiles evacuated raw by DVE (px 0..4608); rest Act+tanh
NCH = 52  # transpose chunks: px 0..6656
RAW_CH = 36  # chunks from the raw (DVE) region (px 0..4608); rest tanh-ed
GBASE = NCH * 128  # 6656: pool route start
GEND = PXS - 2 * HB  # 16128: junk rows excluded
G = GEND - GBASE  # 9472
GW = G // 128  # 74
WAVE_SZ = 8  # chunks per transpose/tree wave (1 PSUM bank)
TAIL_LAG = 2  # images between a block and its finals/output DMAs
F8 = mybir.dt.float8e4
F16 = mybir.dt.float16
F32 = mybir.dt.float32
DR = mybir.MatmulPerfMode.DoubleRow
AF = mybir.ActivationFunctionType


def build_kernel(reps=1, timing=False):
    """reps > 1 repeats the whole per-core compute in one NEFF (HW timing).

    timing=True declares the big image input and outputs as Internal DRAM
    (zero-initialized on device) so per-call host<->device transfer is tiny."""
    nc = bacc.Bacc(trn_type="TRN2", target_bir_lowering=False, debug=False)
    io_kind = "Internal" if timing else None
    px_d = nc.dram_tensor(
        "px", [IMGS_PER_CORE, 128, PITCH], F8, kind=io_kind or "ExternalInput"
    )
    wp = nc.dram_tensor("wp", [128, 6, 128], F8, kind="ExternalInput")
    bias = nc.dram_tensor("bias", [128, 1], F32, kind="ExternalInput")
    ident = nc.dram_tensor("ident", [128, 128], F16, kind="ExternalInput")
    outp = nc.dram_tensor(
        "outp", [IMGS_PER_CORE, 128, GW], F16, kind=io_kind or "ExternalOutput"
    )
    outt = nc.dram_tensor(
        "outt", [IMGS_PER_CORE, 128, NCH], F16, kind=io_kind or "ExternalOutput"
    )
    sink = (
        nc.dram_tensor("sink", [1, 64], F32, kind="ExternalOutput") if timing else None
    )

    with tile.TileContext(nc) as tc:
        with (
            tc.tile_pool(name="consts", bufs=1) as consts,
            tc.tile_pool(name="dpool", bufs=2) as dpool,
            tc.tile_pool(name="mpool", bufs=2) as mpool,
            tc.tile_pool(name="rpool", bufs=3) as rpool,
            tc.tile_pool(name="spool", bufs=3) as spool,
            tc.tile_pool(name="fpool", bufs=2) as fpool,
            tc.tile_pool(name="gpool", bufs=4) as gpool,
            tc.tile_pool(name="pcpool", bufs=2, space="PSUM") as pcpool,
            tc.tile_pool(name="tpool", bufs=2, space="PSUM") as tpool,
        ):
            wpt = consts.tile([128, 6, 128], F8)
            nc.gpsimd.dma_start(out=wpt[:], in_=wp.ap())
            bt = consts.tile([128, 1], F32)
            nc.gpsimd.dma_start(out=bt[:], in_=bias.ap())
            idt = consts.tile([128, 128], F16)
            nc.gpsimd.dma_start(out=idt[:], in_=ident.ap())

            if timing:
                z = dpool.tile([128, PITCH], F8, tag="px")
                for q in range(2):
                    nc.vector.memset(z[:, q * 8320 : (q + 1) * 8320], 0.0)
                for img in range(IMGS_PER_CORE):
                    nc.sync.dma_start(out=px_d.ap()[img], in_=z[:])
                zs = fpool.tile([1, 64], F32, tag="sink")
                nc.vector.memset(zs[:], 0.0)
                nc.sync.dma_start(out=sink.ap(), in_=zs[:])

            def emit_finals(state):
                # lag-2: tanh finals + output DMAs on the Act queue; the
                # scatter (Pool-dispatched, right after the all_reduce)
                # completed ~2 images ago.
                img, rs, mall = state
                f2 = fpool.tile([128, GW], F16, tag="f2")
                nc.scalar.activation(out=f2[:], in_=rs[:], func=AF.Tanh, scale=-1.0)
                nc.gpsimd.dma_start(out=outp.ap()[img], in_=f2[:])
                g2 = gpool.tile([128, NCH], F16, tag="g2")
                g1 = gpool.tile([128, NCH], F16, tag="g1")
                nc.scalar.activation(
                    out=g1[:, 0:RAW_CH], in_=mall[:, 0:RAW_CH],
                    func=AF.Tanh, scale=-1.0,
                )
                nc.scalar.activation(
                    out=g2[:, 0:RAW_CH], in_=g1[:, 0:RAW_CH], func=AF.Tanh
                )
                nc.scalar.activation(
                    out=g2[:, RAW_CH:NCH], in_=mall[:, RAW_CH:NCH],
                    func=AF.Tanh, scale=-1.0,
                )
                nc.gpsimd.dma_start(out=outt.ap()[img], in_=g2[:])

            tails = []
            for img in [i for _ in range(reps) for i in range(IMGS_PER_CORE)]:
                pxt = dpool.tile([128, PITCH], F8, tag="px")
                nc.sync.dma_start(out=pxt[:], in_=px_d.ap()[img])
                pxt_t = pxt[:].tensor

                m = mpool.tile([128, PXS], F16, tag="m")
                mall = gpool.tile([128, NCH], F16, tag="mall")
                next_wave = 0
                for t, (off, npx) in enumerate(zip(TILE_OFF, TILE_PX)):
                    nsub = npx // 512
                    pcb = pcpool.tile([128, 3, 512], F32, tag="pc")
                    for j in range(nsub):
                        for c in range(3):
                            rhs = bass.AP(
                                tensor=pxt_t,
                                offset=off + 512 * j + c,
                                ap=[[PITCH, 128], [HB, 2], [1, 512]],
                            )
                            nc.tensor.matmul(
                                pcb[:, j, :],
                                lhsT=wpt[:, 2 * c : 2 * c + 2, :],
                                rhs=rhs,
                                start=(c == 0),
                                stop=(c == 2),
                                perf_mode=DR,
                            )
                    if t < DVE_TILES:
                        nc.vector.tensor_scalar_add(
                            out=m[:, off : off + npx],
                            in0=pcb[:, 0:nsub, :],
                            scalar1=bt[:],
                        )
                    else:
                        nc.scalar.activation(
                            out=m[:, off : off + npx],
                            in_=pcb[:, 0:nsub, :],
                            func=AF.Tanh,
                            bias=bt[:],
                        )
                    # interleave transpose+tree waves once their source px
                    # (plus one-tile slack) are evacuated
                    while (
                        next_wave * WAVE_SZ < NCH
                        and min(NCH, (next_wave + 1) * WAVE_SZ) * 128
                        <= off + npx - 1024
                    ):
                        ch0 = next_wave * WAVE_SZ
                        wv = min(WAVE_SZ, NCH - ch0)
                        tp = tpool.tile([128, WAVE_SZ, 128], F16, tag="tp")
                        for jj in range(wv):
                            c0 = (ch0 + jj) * 128
                            nc.tensor.transpose(
                                out=tp[:, jj, :],
                                in_=m[:, c0 : c0 + 128],
                                identity=idt[:],
                            )
                        nc.vector.tensor_reduce(
                            out=mall[:, ch0 : ch0 + wv],
                            in_=tp[:, 0:wv, :],
                            axis=mybir.AxisListType.X,
                            op=mybir.AluOpType.max,
                        )
                        next_wave += 1

                # --- Pool route: px GBASE..GEND (tanh-ed at evac) ---
                r = rpool.tile([128, G], F16, tag="r")
                nc.gpsimd.partition_all_reduce(
                    r[:], m[:, GBASE:GEND], channels=128,
                    reduce_op=bass_isa.ReduceOp.max,
                )

                # scatter row 0 of r across partitions, SBUF->SBUF, on the
                # Pool queue (zero wait: directly after its all_reduce)
                rs = spool.tile([128, GW], F16, tag="rs")
                nc.gpsimd.dma_start(out=rs[:], in_=r[0:1, 0:G])

                tails.append((img, rs, mall))
                if len(tails) > TAIL_LAG:
                    emit_finals(tails.pop(0))
            for state in tails:
                emit_finals(state)
    nc.compile()
    return nc


def prep_inputs(x, weight, bias):
    """Host-side packing -> per-core input maps (list of 8 dicts)."""
    x = np.asarray(x, dtype=np.float32)
    weight = np.asarray(weight, dtype=np.float32)
    bias = np.asarray(bias, dtype=np.float32)
    f8np = mybir.dt.np(F8)

    # px layout [N, 128, NBLK, HB]: p<64 -> x[c=p, row b]; p>=64 -> x[c=p-64, b+1]
    px = np.zeros((N_IMGS, 128, NBLK, HB), dtype=np.float32)
    px[:, 0:64, 0:H, :] = x
    px[:, 64:128, 0 : H - 1, :] = x[:, :, 1:, :]
    px = px.reshape(N_IMGS, 128, PITCH).astype(f8np)

    wneg = -weight  # negate so channel-min becomes max
    wp = np.zeros((128, 6, 128), dtype=np.float32)
    for c in range(3):
        wp[0:64, 2 * c] = wneg[:, :, 0, c].T
        wp[64:128, 2 * c] = wneg[:, :, 1, c].T
        wp[64:128, 2 * c + 1] = wneg[:, :, 2, c].T
    wp = wp.astype(f8np)

    b2 = -bias.reshape(128, 1).astype(np.float32)
    idn = np.eye(128, dtype=np.float16)

    in_maps = []
    for core in range(N_CORES):
        sl = slice(core * IMGS_PER_CORE, (core + 1) * IMGS_PER_CORE)
        in_maps.append(
            {
                "px": np.ascontiguousarray(px[sl]),
                "wp": wp,
                "bias": b2,
                "ident": idn,
            }
        )
    return in_maps


def timing_in_maps():
    f8np = mybir.dt.np(F8)
    return [
        {
            "wp": np.zeros((128, 6, 128), dtype=f8np),
            "bias": np.zeros((128, 1), dtype=np.float32),
            "ident": np.eye(128, dtype=np.float16),
        }
    ] * N_CORES


def assemble_output(results):
    """results: list of 8 per-core out dicts -> full [32, 1, 126, 126] f32."""
    full = np.zeros((N_IMGS, PXS), dtype=np.float32)
    for core in range(N_CORES):
        op = np.asarray(results[core]["outp"], dtype=np.float32)  # [4, 128, GW]
        ot = np.asarray(results[core]["outt"], dtype=np.float32)  # [4, 128, NCH]
        for i in range(IMGS_PER_CORE):
            img = core * IMGS_PER_CORE + i
            full[img, 0:GBASE] = ot[i].T.reshape(-1)  # px = c*128 + p
            full[img, GBASE:GEND] = op[i].reshape(-1)  # px = GBASE + p*GW + j
    out = full.reshape(N_IMGS, NROW, HB)[:, 0:HO, 0:WO]
    return np.ascontiguousarray(out.reshape(N_IMGS, 1, HO, WO))


_NC_CACHE = None


def kernel(x, weight, bias):
    global _NC_CACHE
    if _NC_CACHE is None:
        _NC_CACHE = build_kernel()
    in_maps = prep_inputs(x, weight, bias)
    res = run_bass_kernel_spmd(_NC_CACHE, in_maps, list(range(N_CORES)))
    return assemble_output(res.results)
